# revision 66
# baseline (speedup 1.0000x reference)
"""Self-attention (SAGAN-style) Trainium2 kernel, data-parallel over batch on
8 NeuronCores (2 images per core, no collectives).

Reference computation per batch image (B=16, H=W=64, C=512):
    f = x @ Wf                         [4096, 64]   queries
    xp = avgpool2x2(x)                 [1024, 512]
    g = xp @ Wg                        [1024, 64]   keys
    h = xp @ Wh                        [1024, 256]  values
    a = softmax(f @ g^T, axis=-1)      [4096, 1024]
    out = (a @ h) @ Wo + x             [4096, 512]

Per-core dataflow (software-pipelined across the 2 images):
  - x cast-loaded f32->bf16 by SWDGE DMA in [128, 2048] groups, PE-transposed
    (regular matmul vs identity, bf16) to xT [c,q]; 2x2 sum-pooling runs
    incrementally per q-group via strided adds (w-pairs on DVE, h-pairs on
    GPSIMD); Wg/Wh are pre-scaled 0.25 on host so sum-pool == avg-pool.
  - Projections (bf16): f2T [d dup2, q] (lhsT = [Wf|Wf]), g2T [d dup2, k],
    h [k, e]. The d=64 score matmuls are row-packed two-at-a-time into the
    128x128 PE array via tile_position (the duplication feeds rows 64-127).
  - Scores sT = g2T^T f2T accumulate in [k, q] layout; exp on ACT reads PSUM
    directly and writes fp8e4 with a free bias of -4*ln2 (softmax-invariant,
    keeps exp outputs inside fp8e4's +-240 range; no max-subtraction needed
    since |s| <= ~6.2).
  - Z[q] = sum_k exp via matmul(lhsT=exp chunk, rhs=const[128,1]) accumulated
    over k chunks -- lands [q-partition, 1], the orientation the epilogue
    needs. The const is 8.0 = alpha*beta*gamma, pre-compensating the fp8
    scale factors below so no extra scaling op exists anywhere.
  - yT = h^T exp and out_pre = yT^T Wo both run as fp8e4 DoubleRow matmuls
    (2 fp8 weights/cell, 2x MACs): h is evacuated as 2*h (alpha), yT as
    0.25*yT (gamma), Wo is host-scaled 16x (beta) to center fp8 dynamic
    range; all three factors cancel exactly through 1/Z.
  - Epilogue: one DVE scalar_tensor_tensor does out = po * (1/Z) + x.
  - Batch 1's load/transpose/pool units are emitted inside batch 0's span
    loop so they fill engine gaps (engines execute their streams in order).
"""

import numpy as np

B, H, W, C = 16, 64, 64, 512
NCORES = 8
BPC = B // NCORES          # batches per core
HW = H * W                 # 4096 queries
KP = HW // 4               # 1024 pooled keys
D2 = 128                   # duplicated query/key dim (2 x 64)
E = C // 2                 # 256 value dim
P = 128

N_QC = HW // P             # 32 q chunks of 128
N_SPAN = 8                 # q spans of 512
N_CC = C // P              # 4 channel chunks
N_KC = KP // P             # 8 key chunks

ROWPACK = True


def build_nc():
    from contextlib import ExitStack
    import concourse.bacc as bacc
    import concourse.mybir as mybir
    from concourse.tile import TileContext

    fp32 = mybir.dt.float32
    bf16 = mybir.dt.bfloat16
    fp8 = mybir.dt.float8e4
    AF = mybir.ActivationFunctionType
    ALU = mybir.AluOpType

    nc = bacc.Bacc("TRN2", target_bir_lowering=False, debug=False,
                   num_devices=NCORES)
    x_ext = nc.dram_tensor("x", [BPC, HW, C], fp32, kind="ExternalInput").ap()
    wf2_ext = nc.dram_tensor("wf2", [C, P], fp32, kind="ExternalInput").ap()
    wg2_ext = nc.dram_tensor("wg2", [C, P], fp32, kind="ExternalInput").ap()
    wh_ext = nc.dram_tensor("wh", [C, E], fp32, kind="ExternalInput").ap()
    wo_ext = nc.dram_tensor("wo", [E, C], fp32, kind="ExternalInput").ap()
    ident_ext = nc.dram_tensor("ident", [P, P], fp32, kind="ExternalInput").ap()
    out_ext = nc.dram_tensor("out", [BPC, HW, C], fp32, kind="ExternalOutput").ap()

    with ExitStack() as ctx:
        tc = ctx.enter_context(TileContext(nc))

        const = ctx.enter_context(tc.tile_pool(name="const", bufs=1))
        ident = const.tile([P, P], bf16)
        ident_f = const.tile([P, P], fp32)
        nc.sync.dma_start(out=ident_f[:], in_=ident_ext[:])
        nc.vector.tensor_copy(ident[:], ident_f[:])
        ones = const.tile([P, 2], fp8)
        nc.vector.memset(ones[:], 8.0)
        ebias = const.tile([P, 1], fp32)
        nc.vector.memset(ebias[:], -2.772588722239781)

        wf2 = const.tile([P, 4 * P], bf16)
        wg2 = const.tile([P, 4 * P], bf16)
        whb = const.tile([P, 4 * E], bf16)
        wob = const.tile([P, 2 * C], fp8)
        wst_pool = ctx.enter_context(tc.tile_pool(name="wst", bufs=4))

        def wload(dst_slice, src_slice, n):
            st = wst_pool.tile([P, n], fp32, tag="wst", name="wst")
            nc.sync.dma_start(out=st[:], in_=src_slice)
            nc.vector.tensor_copy(dst_slice, st[:])

        def emit_weight_loads():
            for cc in range(N_CC):
                wload(wf2[:, cc * P:(cc + 1) * P],
                      wf2_ext[cc * P:(cc + 1) * P, :], P)
                wload(wg2[:, cc * P:(cc + 1) * P],
                      wg2_ext[cc * P:(cc + 1) * P, :], P)
                wload(whb[:, cc * E:(cc + 1) * E],
                      wh_ext[cc * P:(cc + 1) * P, :], E)
            for ec in range(2):
                wload(wob[:, ec * C:(ec + 1) * C],
                      wo_ext[ec * P:(ec + 1) * P, :], C)

        xb_pool = ctx.enter_context(tc.tile_pool(name="xb", bufs=16))
        xT_pool = ctx.enter_context(tc.tile_pool(name="xT", bufs=5))
        xpT_pool = ctx.enter_context(tc.tile_pool(name="xpT", bufs=5))
        ptmp_pool = ctx.enter_context(tc.tile_pool(name="ptmp", bufs=2))
        f2T_pool = ctx.enter_context(tc.tile_pool(name="f2T", bufs=10))
        g2T_pool = ctx.enter_context(tc.tile_pool(name="g2T", bufs=3))
        h_pool = ctx.enter_context(tc.tile_pool(name="hkb", bufs=10))
        es_pool = ctx.enter_context(tc.tile_pool(name="es", bufs=14))
        yT_pool = ctx.enter_context(tc.tile_pool(name="yT", bufs=6))
        rz_pool = ctx.enter_context(tc.tile_pool(name="rz", bufs=6))
        o_pool = ctx.enter_context(tc.tile_pool(name="o", bufs=8))
        pbank = ctx.enter_context(tc.tile_pool(name="pbank", bufs=4, space="PSUM"))
        psS = ctx.enter_context(tc.tile_pool(name="psS", bufs=2, space="PSUM"))

        # per-batch tile state
        S = [dict(xg=[], xT=[], xpT=[], f2T=[], g2T=[], hk=[], es={})
             for _ in range(BPC)]

        def emit_A_load(b, qg):
            """Issue the cast-load DMA for one q-group."""
            st = S[b]
            if qg == 0:
                for cc in range(N_CC):
                    st["xT"].append(
                        xT_pool.tile([P, HW], bf16, tag="xT", name=f"xT{cc}"))
                    st["xpT"].append(
                        xpT_pool.tile([P, KP], bf16, tag="xpT", name=f"xpT{cc}"))
            xgt = xb_pool.tile([P, 4 * C], bf16, tag="xb", name=f"xb{qg}")
            src = x_ext[b, qg * 512:(qg + 1) * 512, :].rearrange(
                "(j p) c -> p j c", p=P)
            nc.gpsimd.dma_start(
                out=xgt.rearrange("p (j c) -> p j c", j=4), in_=src)
            st["xg"].append(xgt)

        def emit_A_unit(b, qg):
            """Transpose + pool + f2T for one loaded q-group."""
            st = S[b]
            xgt = st["xg"][qg]
            for cc in range(N_CC):
                pt = pbank.tile([P, 512], fp32, tag="pb", name="pb")
                for j in range(4):
                    nc.tensor.matmul(
                        pt[:, j * P:(j + 1) * P],
                        lhsT=xgt[:, j * C + cc * P:j * C + (cc + 1) * P],
                        rhs=ident[:],
                        start=True, stop=True)
                dst = st["xT"][cc][:, qg * 512:(qg + 1) * 512]
                nc.scalar.activation(dst, pt[:], AF.Copy)
                # incremental pool of this q-group: 512 q -> 128 k
                # q-span = 8 rows (h) x 64 cols (w)
                v = st["xT"][cc][:, qg * 512:(qg + 1) * 512].rearrange(
                    "p (h w2 t) -> p (h w2) t", w2=32, t=2)
                t1 = ptmp_pool.tile([P, 256], bf16, tag="ptmp", name="ptmp")
                nc.vector.tensor_add(t1[:], v[:, :, 0], v[:, :, 1])
                r2 = t1.rearrange("p (h2 t w) -> p h2 t w", t=2, w=32)
                nc.gpsimd.tensor_add(
                    st["xpT"][cc][:, qg * P:(qg + 1) * P].rearrange(
                        "p (h2 w) -> p h2 w", w=32),
                    r2[:, :, 0, :], r2[:, :, 1, :])
            # f2T for this q-span (only needs this qg's xT columns)
            qs = qg
            xT = st["xT"]
            pf = pbank.tile([P, 512], fp32, tag="pb", name="pb")
            for cc in range(N_CC):
                nc.tensor.matmul(
                    pf[:],
                    lhsT=wf2[:, cc * P:(cc + 1) * P],
                    rhs=xT[cc][:, qs * 512:(qs + 1) * 512],
                    start=(cc == 0), stop=(cc == N_CC - 1))
            ft = f2T_pool.tile([P, 512], bf16, tag="f2T", name=f"f2T{qs}")
            nc.vector.tensor_copy(ft[:], pf[:])
            st["f2T"].append(ft)

        def emit_C_half(b, ks):
            """Projections for one k-half: g2T[ks] + h[kc 4ks..4ks+3].
            Only needs q-groups 4ks..4ks+3 pooled, so the first half can be
            emitted right after A-unit 3 -- unblocking every span's first
            four score/exp chunks four q-groups earlier."""
            st = S[b]
            xT, xpT = st["xT"], st["xpT"]
            pg = pbank.tile([P, 512], fp32, tag="pb", name="pb")
            for cc in range(N_CC):
                nc.tensor.matmul(
                    pg[:],
                    lhsT=wg2[:, cc * P:(cc + 1) * P],
                    rhs=xpT[cc][:, ks * 512:(ks + 1) * 512],
                    start=(cc == 0), stop=(cc == N_CC - 1))
            gt = g2T_pool.tile([P, 512], bf16, tag="g2T", name=f"g2T{ks}")
            nc.scalar.activation(gt[:], pg[:], AF.Copy)
            st["g2T"].append(gt)
            for pr in range(2 * ks, 2 * ks + 2):
                ph = pbank.tile([P, 2 * E], fp32, tag="pb", name="ph")
                for half in range(2):
                    kc = pr * 2 + half
                    for cc in range(N_CC):
                        nc.tensor.matmul(
                            ph[:, half * E:(half + 1) * E],
                            lhsT=xpT[cc][:, kc * P:(kc + 1) * P],
                            rhs=whb[:, cc * E:(cc + 1) * E],
                            start=(cc == 0), stop=(cc == N_CC - 1))
                ht = h_pool.tile([P, 2 * E], fp8, tag="hkb", name=f"hkb{pr}")
                st["hk"].append(ht)
                nc.vector.tensor_scalar_mul(ht[:], ph[:], 2.0)

        def emit_span_scores(b, qs, kh):
            """sT + exp for kc pairs (2kh, 2kh+1) of span qs. kh=0 only
            needs g2T[0] (first 512 keys), so it can prefetch into the
            stage-A ramp where psS and ACT are otherwise idle."""
            st = S[b]
            f2T, g2T = st["f2T"], st["g2T"]
            sdict = st["es"].setdefault(qs, {})
            for kp_i in (2 * kh, 2 * kh + 1):
                ps = psS.tile([P, 1024], fp32, tag="psS", name="psS")
                for half in range(2):
                    kc = kp_i * 2 + half
                    ks, off = kc // 4, (kc % 4) * P
                    if ROWPACK:
                        rlo = 64 * (kc % 2)
                        tp = (rlo, 0)
                        lhsT = g2T[ks][rlo:rlo + 64, off:off + P]
                        rhs = f2T[qs][rlo:rlo + 64, :]
                        nc.tensor.matmul(
                            ps[:, half * 512:(half + 1) * 512],
                            lhsT=lhsT, rhs=rhs,
                            start=True, stop=True, tile_position=tp)
                    else:
                        nc.tensor.matmul(
                            ps[:, half * 512:(half + 1) * 512],
                            lhsT=g2T[ks][0:64, off:off + P],
                            rhs=f2T[qs][0:64, :],
                            start=True, stop=True)
                et = es_pool.tile([P, 1024], fp8, tag="es", name="es")
                nc.scalar.activation(et[:], ps[:], AF.Exp,
                                     bias=ebias[:])
                sdict[kp_i] = et

        def emit_span(b, qs, pre_kh0=False):
            st = S[b]
            hk, xg = st["hk"], st["xg"]
            if True:
                if not pre_kh0:
                    emit_span_scores(b, qs, 0)
                emit_span_scores(b, qs, 1)
                es = [st["es"][qs][i] for i in range(4)]
                del st["es"][qs]

                # D3: Z[q] per q-chunk via matmul(lhsT=exp chunk, rhs=ones).
                # Plain fp8 (not DoubleRow): at FD=1 these are LDWEIGHTS-bound
                # and FWL (4x fp8 weight load) beats DoubleRow's 2x-wide
                # FWL-less load.
                pz = pbank.tile([P, 4], fp32, tag="pb", name="pz")
                for kc in range(N_KC):
                    for q4 in range(4):
                        lhsT = es[kc // 2][:, (kc % 2) * 512 + q4 * P:
                                           (kc % 2) * 512 + (q4 + 1) * P]
                        nc.tensor.matmul(
                            pz[:, q4:q4 + 1], lhsT=lhsT,
                            rhs=ones[:, 0:1],
                            start=(kc == 0), stop=(kc == N_KC - 1))
                rz = rz_pool.tile([P, 4], fp32, tag="rz", name="rz")
                nc.vector.reciprocal(rz[:], pz[:])

                # D4: yT[e, q_span] = h^T @ expsT  (fp8 DoubleRow, k pairs)
                yt = yT_pool.tile([P, 1024], fp8, tag="yT", name="yT")
                for ec in range(2):
                    py = pbank.tile([P, 512], fp32, tag="pb", name="pb")
                    for pr in range(4):
                        h3 = hk[pr].rearrange("p (ko e) -> p ko e", ko=2)
                        e3 = es[pr].rearrange("p (ko q) -> p ko q", ko=2)
                        nc.tensor.matmul(
                            py[:],
                            lhsT=h3[:, :, ec * P:(ec + 1) * P],
                            rhs=e3[:, :, :],
                            start=(pr == 0), stop=(pr == 3),
                            perf_mode=mybir.MatmulPerfMode.DoubleRow)
                    nc.vector.tensor_scalar_mul(
                        yt[:, ec * 512:(ec + 1) * 512], py[:], 0.25)

                # D5+D6: out[q, c] = (yT^T @ Wo) * (1/Z) + x, then DMA out
                y3 = yt.rearrange("p (ko q) -> p ko q", ko=2)
                w3 = wob.rearrange("p (ko c) -> p ko c", ko=2)
                for q4 in range(4):
                    qc = qs * 4 + q4
                    po = pbank.tile([P, 512], fp32, tag="pb", name="pb")
                    nc.tensor.matmul(
                        po[:],
                        lhsT=y3[:, :, q4 * P:(q4 + 1) * P],
                        rhs=w3[:, :, :],
                        start=True, stop=True,
                        perf_mode=mybir.MatmulPerfMode.DoubleRow)
                    ot = o_pool.tile([P, C], fp32, tag="o", name="ot")
                    xres = xg[qc // 4][:, (qc % 4) * C:(qc % 4 + 1) * C]
                    nc.vector.scalar_tensor_tensor(
                        out=ot[:], in0=po[:], scalar=rz[:, q4:q4 + 1],
                        in1=xres, op0=ALU.mult, op1=ALU.add)
                    nc.sync.dma_start(
                        out=out_ext[b, qc * P:(qc + 1) * P, :], in_=ot[:])

        # software-pipelined emission: loads run 3 q-groups ahead of their
        # compute; batch 1's stage A rides inside batch 0's span loop so its
        # loads/transposes/pools fill engine gaps
        emit_A_load(0, 0)
        emit_A_load(0, 1)
        emit_A_load(0, 2)
        emit_weight_loads()
        for qg in range(8):
            if qg + 3 < 8:
                emit_A_load(0, qg + 3)
            emit_A_unit(0, qg)
            if qg == 3:
                emit_C_half(0, 0)
        emit_C_half(0, 1)
        emit_A_load(1, 0)
        emit_A_load(1, 1)
        for qs in range(N_SPAN):
            if qs + 2 < N_SPAN:
                emit_A_load(1, qs + 2)
            emit_A_unit(1, qs)
            if qs == 3:
                emit_C_half(1, 0)
            if qs == 7:
                emit_C_half(1, 1)
            emit_span(0, qs)
        for qs in range(N_SPAN):
            emit_span(1, qs)

    nc.compile()
    return nc


_NC_CACHE = {}


def _get_nc():
    if "nc" not in _NC_CACHE:
        _NC_CACHE["nc"] = build_nc()
    return _NC_CACHE["nc"]


def _make_in_maps(inputs):
    x = np.ascontiguousarray(np.asarray(inputs["x"], dtype=np.float32))
    Wf = np.asarray(inputs["Wf"], dtype=np.float32)
    Wg = np.asarray(inputs["Wg"], dtype=np.float32)
    Wh = np.asarray(inputs["Wh"], dtype=np.float32)
    Wo = np.asarray(inputs["Wo"], dtype=np.float32)

    xr = x.reshape(B, HW, C)
    wf2 = np.ascontiguousarray(np.concatenate([Wf, Wf], axis=1))
    wg2 = np.ascontiguousarray(np.concatenate([Wg, Wg], axis=1) * 0.25)
    whq = np.ascontiguousarray(Wh * 0.25)
    wo = np.ascontiguousarray(Wo * 16.0)

    ident = np.eye(P, dtype=np.float32)
    return [
        {"x": np.ascontiguousarray(xr[i * BPC:(i + 1) * BPC]),
         "wf2": wf2, "wg2": wg2, "wh": whq, "wo": wo, "ident": ident}
        for i in range(NCORES)
    ]


def run(inputs, trace=False, **kw):
    from concourse.bass_utils import run_bass_kernel_spmd
    nc = _get_nc()
    in_maps = _make_in_maps(inputs)
    res = run_bass_kernel_spmd(nc, in_maps, core_ids=list(range(NCORES)),
                               trace=trace, **kw)
    out = np.concatenate([r["out"] for r in res.results], axis=0)
    return out.reshape(B, H, W, C).astype(np.float32), res


def kernel(**inputs):
    out, _ = run(inputs, trace=False)
    return out


# revision 72
# speedup vs baseline: 1.0125x; 1.0125x over previous
"""Self-attention (SAGAN-style) Trainium2 kernel, data-parallel over batch on
8 NeuronCores (2 images per core, no collectives).

Reference computation per batch image (B=16, H=W=64, C=512):
    f = x @ Wf                         [4096, 64]   queries
    xp = avgpool2x2(x)                 [1024, 512]
    g = xp @ Wg                        [1024, 64]   keys
    h = xp @ Wh                        [1024, 256]  values
    a = softmax(f @ g^T, axis=-1)      [4096, 1024]
    out = (a @ h) @ Wo + x             [4096, 512]

Per-core dataflow (software-pipelined across the 2 images):
  - x cast-loaded f32->bf16 by SWDGE DMA in [128, 2048] groups, PE-transposed
    (regular matmul vs identity, bf16) to xT [c,q]; 2x2 sum-pooling runs
    incrementally per q-group via strided adds (w-pairs on DVE, h-pairs on
    GPSIMD); Wg/Wh are pre-scaled 0.25 on host so sum-pool == avg-pool.
  - Projections (bf16): f2T [d dup2, q] (lhsT = [Wf|Wf]), g2T [d dup2, k],
    h [k, e]. The d=64 score matmuls are row-packed two-at-a-time into the
    128x128 PE array via tile_position (the duplication feeds rows 64-127).
  - Scores sT = g2T^T f2T accumulate in [k, q] layout; exp on ACT reads PSUM
    directly and writes fp8e4 with a free bias of -4*ln2 (softmax-invariant,
    keeps exp outputs inside fp8e4's +-240 range; no max-subtraction needed
    since |s| <= ~6.2).
  - Z[q] = sum_k exp via matmul(lhsT=exp chunk, rhs=const[128,1]) accumulated
    over k chunks -- lands [q-partition, 1], the orientation the epilogue
    needs. The const is 8.0 = alpha*beta*gamma, pre-compensating the fp8
    scale factors below so no extra scaling op exists anywhere.
  - yT = h^T exp and out_pre = yT^T Wo both run as fp8e4 DoubleRow matmuls
    (2 fp8 weights/cell, 2x MACs): h is evacuated as 2*h (alpha), yT as
    0.25*yT (gamma), Wo is host-scaled 16x (beta) to center fp8 dynamic
    range; all three factors cancel exactly through 1/Z.
  - Epilogue: one DVE scalar_tensor_tensor does out = po * (1/Z) + x.
  - Batch 1's load/transpose/pool units are emitted inside batch 0's span
    loop so they fill engine gaps (engines execute their streams in order).
"""

import numpy as np

B, H, W, C = 16, 64, 64, 512
NCORES = 8
BPC = B // NCORES          # batches per core
HW = H * W                 # 4096 queries
KP = HW // 4               # 1024 pooled keys
D2 = 128                   # duplicated query/key dim (2 x 64)
E = C // 2                 # 256 value dim
P = 128

N_QC = HW // P             # 32 q chunks of 128
N_SPAN = 8                 # q spans of 512
N_CC = C // P              # 4 channel chunks
N_KC = KP // P             # 8 key chunks

ROWPACK = True


def build_nc():
    from contextlib import ExitStack
    import concourse.bacc as bacc
    import concourse.mybir as mybir
    from concourse.tile import TileContext

    fp32 = mybir.dt.float32
    bf16 = mybir.dt.bfloat16
    fp8 = mybir.dt.float8e4
    AF = mybir.ActivationFunctionType
    ALU = mybir.AluOpType

    nc = bacc.Bacc("TRN2", target_bir_lowering=False, debug=False,
                   num_devices=NCORES)
    x_ext = nc.dram_tensor("x", [BPC, HW, C], fp32, kind="ExternalInput").ap()
    wf2_ext = nc.dram_tensor("wf2", [C, P], fp32, kind="ExternalInput").ap()
    wg2_ext = nc.dram_tensor("wg2", [C, P], fp32, kind="ExternalInput").ap()
    wh_ext = nc.dram_tensor("wh", [C, E], fp32, kind="ExternalInput").ap()
    wo_ext = nc.dram_tensor("wo", [E, C], fp32, kind="ExternalInput").ap()
    ident_ext = nc.dram_tensor("ident", [P, P], fp32, kind="ExternalInput").ap()
    out_ext = nc.dram_tensor("out", [BPC, HW, C], fp32, kind="ExternalOutput").ap()

    with ExitStack() as ctx:
        tc = ctx.enter_context(TileContext(nc))

        const = ctx.enter_context(tc.tile_pool(name="const", bufs=1))
        ident = const.tile([P, P], bf16)
        ident_f = const.tile([P, P], fp32)
        nc.sync.dma_start(out=ident_f[:], in_=ident_ext[:])
        nc.vector.tensor_copy(ident[:], ident_f[:])
        ones = const.tile([P, 2], fp8)
        nc.vector.memset(ones[:], 8.0)
        ebias = const.tile([P, 1], fp32)
        nc.vector.memset(ebias[:], -2.772588722239781)

        wf2 = const.tile([P, 4 * P], bf16)
        wg2 = const.tile([P, 4 * P], bf16)
        whb = const.tile([P, 4 * E], bf16)
        wob = const.tile([P, 2 * C], fp8)
        wst_pool = ctx.enter_context(tc.tile_pool(name="wst", bufs=4))

        def wload(dst_slice, src_slice, n):
            st = wst_pool.tile([P, n], fp32, tag="wst", name="wst")
            nc.sync.dma_start(out=st[:], in_=src_slice)
            nc.vector.tensor_copy(dst_slice, st[:])

        def emit_weight_loads():
            for cc in range(N_CC):
                wload(wf2[:, cc * P:(cc + 1) * P],
                      wf2_ext[cc * P:(cc + 1) * P, :], P)
                wload(wg2[:, cc * P:(cc + 1) * P],
                      wg2_ext[cc * P:(cc + 1) * P, :], P)
                wload(whb[:, cc * E:(cc + 1) * E],
                      wh_ext[cc * P:(cc + 1) * P, :], E)
            for ec in range(2):
                wload(wob[:, ec * C:(ec + 1) * C],
                      wo_ext[ec * P:(ec + 1) * P, :], C)

        xb_pool = ctx.enter_context(tc.tile_pool(name="xb", bufs=16))
        xT_pool = ctx.enter_context(tc.tile_pool(name="xT", bufs=5))
        xpT_pool = ctx.enter_context(tc.tile_pool(name="xpT", bufs=5))
        ptmp_pool = ctx.enter_context(tc.tile_pool(name="ptmp", bufs=4))
        f2T_pool = ctx.enter_context(tc.tile_pool(name="f2T", bufs=10))
        g2T_pool = ctx.enter_context(tc.tile_pool(name="g2T", bufs=3))
        h_pool = ctx.enter_context(tc.tile_pool(name="hkb", bufs=10))
        es_pool = ctx.enter_context(tc.tile_pool(name="es", bufs=14))
        yT_pool = ctx.enter_context(tc.tile_pool(name="yT", bufs=6))
        rz_pool = ctx.enter_context(tc.tile_pool(name="rz", bufs=6))
        o_pool = ctx.enter_context(tc.tile_pool(name="o", bufs=8))
        pbank = ctx.enter_context(tc.tile_pool(name="pbank", bufs=4, space="PSUM"))
        psS = ctx.enter_context(tc.tile_pool(name="psS", bufs=2, space="PSUM"))

        # per-batch tile state
        S = [dict(xg=[], xT=[], xpT=[], f2T=[], g2T=[], hk=[], es={})
             for _ in range(BPC)]

        def emit_A_load(b, qg):
            """Issue the cast-load DMA for one q-group."""
            st = S[b]
            if qg == 0:
                for cc in range(N_CC):
                    st["xT"].append(
                        xT_pool.tile([P, HW], bf16, tag="xT", name=f"xT{cc}"))
                    st["xpT"].append(
                        xpT_pool.tile([P, KP], bf16, tag="xpT", name=f"xpT{cc}"))
            xgt = xb_pool.tile([P, 4 * C], bf16, tag="xb", name=f"xb{qg}")
            src = x_ext[b, qg * 512:(qg + 1) * 512, :].rearrange(
                "(j p) c -> p j c", p=P)
            nc.gpsimd.dma_start(
                out=xgt.rearrange("p (j c) -> p j c", j=4), in_=src)
            st["xg"].append(xgt)

        def emit_A_unit(b, qg):
            """Transpose + pool + f2T for one loaded q-group."""
            st = S[b]
            xgt = st["xg"][qg]
            for cc in range(N_CC):
                pt = pbank.tile([P, 512], fp32, tag="pb", name="pb")
                for j in range(4):
                    nc.tensor.matmul(
                        pt[:, j * P:(j + 1) * P],
                        lhsT=xgt[:, j * C + cc * P:j * C + (cc + 1) * P],
                        rhs=ident[:],
                        start=True, stop=True)
                dst = st["xT"][cc][:, qg * 512:(qg + 1) * 512]
                nc.scalar.activation(dst, pt[:], AF.Copy)
                # incremental pool of this q-group: 512 q -> 128 k
                # q-span = 8 rows (h) x 64 cols (w)
                v = st["xT"][cc][:, qg * 512:(qg + 1) * 512].rearrange(
                    "p (h w2 t) -> p (h w2) t", w2=32, t=2)
                t1 = ptmp_pool.tile([P, 256], bf16, tag="ptmp", name="ptmp")
                nc.vector.tensor_add(t1[:], v[:, :, 0], v[:, :, 1])
                r2 = t1.rearrange("p (h2 t w) -> p h2 t w", t=2, w=32)
                nc.gpsimd.tensor_add(
                    st["xpT"][cc][:, qg * P:(qg + 1) * P].rearrange(
                        "p (h2 w) -> p h2 w", w=32),
                    r2[:, :, 0, :], r2[:, :, 1, :])
            # f2T for this q-span (only needs this qg's xT columns)
            qs = qg
            xT = st["xT"]
            pf = pbank.tile([P, 512], fp32, tag="pb", name="pb")
            for cc in range(N_CC):
                nc.tensor.matmul(
                    pf[:],
                    lhsT=wf2[:, cc * P:(cc + 1) * P],
                    rhs=xT[cc][:, qs * 512:(qs + 1) * 512],
                    start=(cc == 0), stop=(cc == N_CC - 1))
            ft = f2T_pool.tile([P, 512], bf16, tag="f2T", name=f"f2T{qs}")
            nc.vector.tensor_copy(ft[:], pf[:])
            st["f2T"].append(ft)

        def emit_C_half(b, ks):
            """Projections for one k-half: g2T[ks] + h[kc 4ks..4ks+3].
            Only needs q-groups 4ks..4ks+3 pooled, so the first half can be
            emitted right after A-unit 3 -- unblocking every span's first
            four score/exp chunks four q-groups earlier."""
            st = S[b]
            xT, xpT = st["xT"], st["xpT"]
            pg = pbank.tile([P, 512], fp32, tag="pb", name="pb")
            for cc in range(N_CC):
                nc.tensor.matmul(
                    pg[:],
                    lhsT=wg2[:, cc * P:(cc + 1) * P],
                    rhs=xpT[cc][:, ks * 512:(ks + 1) * 512],
                    start=(cc == 0), stop=(cc == N_CC - 1))
            gt = g2T_pool.tile([P, 512], bf16, tag="g2T", name=f"g2T{ks}")
            nc.scalar.activation(gt[:], pg[:], AF.Copy)
            st["g2T"].append(gt)
            for pr in range(2 * ks, 2 * ks + 2):
                ph = pbank.tile([P, 2 * E], fp32, tag="pb", name="ph")
                for half in range(2):
                    kc = pr * 2 + half
                    for cc in range(N_CC):
                        nc.tensor.matmul(
                            ph[:, half * E:(half + 1) * E],
                            lhsT=xpT[cc][:, kc * P:(kc + 1) * P],
                            rhs=whb[:, cc * E:(cc + 1) * E],
                            start=(cc == 0), stop=(cc == N_CC - 1))
                ht = h_pool.tile([P, 2 * E], fp8, tag="hkb", name=f"hkb{pr}")
                st["hk"].append(ht)
                nc.vector.tensor_scalar_mul(ht[:], ph[:], 2.0)

        def emit_span_scores(b, qs, kh):
            """sT + exp for kc pairs (2kh, 2kh+1) of span qs. kh=0 only
            needs g2T[0] (first 512 keys), so it can prefetch into the
            stage-A ramp where psS and ACT are otherwise idle."""
            st = S[b]
            f2T, g2T = st["f2T"], st["g2T"]
            sdict = st["es"].setdefault(qs, {})
            for kp_i in (2 * kh, 2 * kh + 1):
                ps = psS.tile([P, 1024], fp32, tag="psS", name="psS")
                for half in range(2):
                    kc = kp_i * 2 + half
                    ks, off = kc // 4, (kc % 4) * P
                    if ROWPACK:
                        rlo = 64 * (kc % 2)
                        tp = (rlo, 0)
                        lhsT = g2T[ks][rlo:rlo + 64, off:off + P]
                        rhs = f2T[qs][rlo:rlo + 64, :]
                        nc.tensor.matmul(
                            ps[:, half * 512:(half + 1) * 512],
                            lhsT=lhsT, rhs=rhs,
                            start=True, stop=True, tile_position=tp)
                    else:
                        nc.tensor.matmul(
                            ps[:, half * 512:(half + 1) * 512],
                            lhsT=g2T[ks][0:64, off:off + P],
                            rhs=f2T[qs][0:64, :],
                            start=True, stop=True)
                et = es_pool.tile([P, 1024], fp8, tag="es", name="es")
                nc.scalar.activation(et[:], ps[:], AF.Exp,
                                     bias=ebias[:])
                sdict[kp_i] = et

        def emit_span(b, qs, pre_kh0=False):
            st = S[b]
            hk, xg = st["hk"], st["xg"]
            if True:
                if not pre_kh0:
                    emit_span_scores(b, qs, 0)
                emit_span_scores(b, qs, 1)
                es = [st["es"][qs][i] for i in range(4)]
                del st["es"][qs]

                # D3: Z[q] per q-chunk via matmul(lhsT=exp chunk, rhs=ones).
                # Plain fp8 (not DoubleRow): at FD=1 these are LDWEIGHTS-bound
                # and FWL (4x fp8 weight load) beats DoubleRow's 2x-wide
                # FWL-less load.
                pz = pbank.tile([P, 4], fp32, tag="pb", name="pz")
                for kc in range(N_KC):
                    for q4 in range(4):
                        lhsT = es[kc // 2][:, (kc % 2) * 512 + q4 * P:
                                           (kc % 2) * 512 + (q4 + 1) * P]
                        nc.tensor.matmul(
                            pz[:, q4:q4 + 1], lhsT=lhsT,
                            rhs=ones[:, 0:1],
                            start=(kc == 0), stop=(kc == N_KC - 1))
                rz = rz_pool.tile([P, 4], fp32, tag="rz", name="rz")
                nc.vector.reciprocal(rz[:], pz[:])

                # D4: yT[e, q_span] = h^T @ expsT  (fp8 DoubleRow, k pairs)
                yt = yT_pool.tile([P, 1024], fp8, tag="yT", name="yT")
                for ec in range(2):
                    py = pbank.tile([P, 512], fp32, tag="pb", name="pb")
                    for pr in range(4):
                        h3 = hk[pr].rearrange("p (ko e) -> p ko e", ko=2)
                        e3 = es[pr].rearrange("p (ko q) -> p ko q", ko=2)
                        nc.tensor.matmul(
                            py[:],
                            lhsT=h3[:, :, ec * P:(ec + 1) * P],
                            rhs=e3[:, :, :],
                            start=(pr == 0), stop=(pr == 3),
                            perf_mode=mybir.MatmulPerfMode.DoubleRow)
                    nc.vector.tensor_scalar_mul(
                        yt[:, ec * 512:(ec + 1) * 512], py[:], 0.25)

                # D5+D6: out[q, c] = (yT^T @ Wo) * (1/Z) + x, then DMA out
                y3 = yt.rearrange("p (ko q) -> p ko q", ko=2)
                w3 = wob.rearrange("p (ko c) -> p ko c", ko=2)
                for q4 in range(4):
                    qc = qs * 4 + q4
                    po = pbank.tile([P, 512], fp32, tag="pb", name="pb")
                    nc.tensor.matmul(
                        po[:],
                        lhsT=y3[:, :, q4 * P:(q4 + 1) * P],
                        rhs=w3[:, :, :],
                        start=True, stop=True,
                        perf_mode=mybir.MatmulPerfMode.DoubleRow)
                    ot = o_pool.tile([P, C], fp32, tag="o", name="ot")
                    xres = xg[qc // 4][:, (qc % 4) * C:(qc % 4 + 1) * C]
                    nc.vector.scalar_tensor_tensor(
                        out=ot[:], in0=po[:], scalar=rz[:, q4:q4 + 1],
                        in1=xres, op0=ALU.mult, op1=ALU.add)
                    nc.sync.dma_start(
                        out=out_ext[b, qc * P:(qc + 1) * P, :], in_=ot[:])

        # software-pipelined emission: loads run 3 q-groups ahead of their
        # compute; batch 1's stage A rides inside batch 0's span loop so its
        # loads/transposes/pools fill engine gaps
        emit_A_load(0, 0)
        emit_A_load(0, 1)
        emit_A_load(0, 2)
        emit_weight_loads()
        for qg in range(8):
            if qg + 3 < 8:
                emit_A_load(0, qg + 3)
            emit_A_unit(0, qg)
            if qg == 3:
                emit_C_half(0, 0)
        emit_C_half(0, 1)
        emit_A_load(1, 0)
        emit_A_load(1, 1)
        for qs in range(N_SPAN):
            if qs + 2 < N_SPAN:
                emit_A_load(1, qs + 2)
            emit_A_unit(1, qs)
            if qs == 3:
                emit_C_half(1, 0)
            if qs == 7:
                emit_C_half(1, 1)
            emit_span(0, qs)
        for qs in range(N_SPAN):
            emit_span(1, qs)

    nc.compile()
    return nc


_NC_CACHE = {}


def _get_nc():
    if "nc" not in _NC_CACHE:
        _NC_CACHE["nc"] = build_nc()
    return _NC_CACHE["nc"]


def _make_in_maps(inputs):
    x = np.ascontiguousarray(np.asarray(inputs["x"], dtype=np.float32))
    Wf = np.asarray(inputs["Wf"], dtype=np.float32)
    Wg = np.asarray(inputs["Wg"], dtype=np.float32)
    Wh = np.asarray(inputs["Wh"], dtype=np.float32)
    Wo = np.asarray(inputs["Wo"], dtype=np.float32)

    xr = x.reshape(B, HW, C)
    wf2 = np.ascontiguousarray(np.concatenate([Wf, Wf], axis=1))
    wg2 = np.ascontiguousarray(np.concatenate([Wg, Wg], axis=1) * 0.25)
    whq = np.ascontiguousarray(Wh * 0.25)
    wo = np.ascontiguousarray(Wo * 16.0)

    ident = np.eye(P, dtype=np.float32)
    return [
        {"x": np.ascontiguousarray(xr[i * BPC:(i + 1) * BPC]),
         "wf2": wf2, "wg2": wg2, "wh": whq, "wo": wo, "ident": ident}
        for i in range(NCORES)
    ]


def run(inputs, trace=False, **kw):
    from concourse.bass_utils import run_bass_kernel_spmd
    nc = _get_nc()
    in_maps = _make_in_maps(inputs)
    res = run_bass_kernel_spmd(nc, in_maps, core_ids=list(range(NCORES)),
                               trace=trace, **kw)
    out = np.concatenate([r["out"] for r in res.results], axis=0)
    return out.reshape(B, H, W, C).astype(np.float32), res


def kernel(**inputs):
    out, _ = run(inputs, trace=False)
    return out


# revision 80
# speedup vs baseline: 1.0327x; 1.0200x over previous
"""Self-attention (SAGAN-style) Trainium2 kernel, data-parallel over batch on
8 NeuronCores (2 images per core, no collectives).

Reference computation per batch image (B=16, H=W=64, C=512):
    f = x @ Wf                         [4096, 64]   queries
    xp = avgpool2x2(x)                 [1024, 512]
    g = xp @ Wg                        [1024, 64]   keys
    h = xp @ Wh                        [1024, 256]  values
    a = softmax(f @ g^T, axis=-1)      [4096, 1024]
    out = (a @ h) @ Wo + x             [4096, 512]

Per-core dataflow (software-pipelined across the 2 images):
  - x cast-loaded f32->bf16 by SWDGE DMA in [128, 2048] groups, PE-transposed
    (regular matmul vs identity, bf16) to xT [c,q]; 2x2 sum-pooling runs
    incrementally per q-group via strided adds (w-pairs on DVE, h-pairs on
    GPSIMD); Wg/Wh are pre-scaled 0.25 on host so sum-pool == avg-pool.
  - Projections (bf16): f2T [d dup2, q] (lhsT = [Wf|Wf]), g2T [d dup2, k],
    h [k, e]. The d=64 score matmuls are row-packed two-at-a-time into the
    128x128 PE array via tile_position (the duplication feeds rows 64-127).
  - Scores sT = g2T^T f2T accumulate in [k, q] layout; exp on ACT reads PSUM
    directly and writes fp8e4 with a free bias of -4*ln2 (softmax-invariant,
    keeps exp outputs inside fp8e4's +-240 range; no max-subtraction needed
    since |s| <= ~6.2).
  - Z[q] = sum_k exp via matmul(lhsT=exp chunk, rhs=const[128,1]) accumulated
    over k chunks -- lands [q-partition, 1], the orientation the epilogue
    needs. The const is 8.0 = alpha*beta*gamma, pre-compensating the fp8
    scale factors below so no extra scaling op exists anywhere.
  - yT = h^T exp and out_pre = yT^T Wo both run as fp8e4 DoubleRow matmuls
    (2 fp8 weights/cell, 2x MACs): h is evacuated as 2*h (alpha), yT as
    0.25*yT (gamma), Wo is host-scaled 16x (beta) to center fp8 dynamic
    range; all three factors cancel exactly through 1/Z.
  - Epilogue: one DVE scalar_tensor_tensor does out = po * (1/Z) + x.
  - Batch 1's load/transpose/pool units are emitted inside batch 0's span
    loop so they fill engine gaps (engines execute their streams in order).
"""

import numpy as np

B, H, W, C = 16, 64, 64, 512
NCORES = 8
BPC = B // NCORES          # batches per core
HW = H * W                 # 4096 queries
KP = HW // 4               # 1024 pooled keys
D2 = 128                   # duplicated query/key dim (2 x 64)
E = C // 2                 # 256 value dim
P = 128

N_QC = HW // P             # 32 q chunks of 128
N_SPAN = 8                 # q spans of 512
N_CC = C // P              # 4 channel chunks
N_KC = KP // P             # 8 key chunks

ROWPACK = True


def build_nc():
    from contextlib import ExitStack
    import concourse.bacc as bacc
    import concourse.mybir as mybir
    from concourse.tile import TileContext

    fp32 = mybir.dt.float32
    bf16 = mybir.dt.bfloat16
    fp8 = mybir.dt.float8e4
    AF = mybir.ActivationFunctionType
    ALU = mybir.AluOpType

    nc = bacc.Bacc("TRN2", target_bir_lowering=False, debug=False,
                   num_devices=NCORES)
    x_ext = nc.dram_tensor("x", [BPC, HW, C], fp32, kind="ExternalInput").ap()
    wf2_ext = nc.dram_tensor("wf2", [C, P], fp32, kind="ExternalInput").ap()
    wg2_ext = nc.dram_tensor("wg2", [C, P], fp32, kind="ExternalInput").ap()
    wh_ext = nc.dram_tensor("wh", [C, E], fp32, kind="ExternalInput").ap()
    wo_ext = nc.dram_tensor("wo", [E, C], fp32, kind="ExternalInput").ap()
    ident_ext = nc.dram_tensor("ident", [P, P], fp32, kind="ExternalInput").ap()
    out_ext = nc.dram_tensor("out", [BPC, HW, C], fp32, kind="ExternalOutput").ap()

    with ExitStack() as ctx:
        tc = ctx.enter_context(TileContext(nc))

        const = ctx.enter_context(tc.tile_pool(name="const", bufs=1))
        ident = const.tile([P, P], bf16)
        ident_f = const.tile([P, P], fp32)
        nc.sync.dma_start(out=ident_f[:], in_=ident_ext[:])
        nc.vector.tensor_copy(ident[:], ident_f[:])
        ones = const.tile([P, 2], fp8)
        nc.vector.memset(ones[:], 8.0)
        ebias = const.tile([P, 1], fp32)
        nc.vector.memset(ebias[:], -2.772588722239781)

        wf2 = const.tile([P, 4 * P], bf16)
        wg2 = const.tile([P, 4 * P], bf16)
        whb = const.tile([P, 4 * E], bf16)
        wob = const.tile([P, 2 * C], fp8)
        wst_pool = ctx.enter_context(tc.tile_pool(name="wst", bufs=4))

        def wload(dst_slice, src_slice, n):
            st = wst_pool.tile([P, n], fp32, tag="wst", name="wst")
            nc.sync.dma_start(out=st[:], in_=src_slice)
            nc.vector.tensor_copy(dst_slice, st[:])

        def emit_weight_loads():
            for cc in range(N_CC):
                wload(wf2[:, cc * P:(cc + 1) * P],
                      wf2_ext[cc * P:(cc + 1) * P, :], P)
                wload(wg2[:, cc * P:(cc + 1) * P],
                      wg2_ext[cc * P:(cc + 1) * P, :], P)
                wload(whb[:, cc * E:(cc + 1) * E],
                      wh_ext[cc * P:(cc + 1) * P, :], E)
            for ec in range(2):
                wload(wob[:, ec * C:(ec + 1) * C],
                      wo_ext[ec * P:(ec + 1) * P, :], C)

        xb_pool = ctx.enter_context(tc.tile_pool(name="xb", bufs=16))
        xT_pool = ctx.enter_context(tc.tile_pool(name="xT", bufs=5))
        xpT_pool = ctx.enter_context(tc.tile_pool(name="xpT", bufs=5))
        ptmp_pool = ctx.enter_context(tc.tile_pool(name="ptmp", bufs=4))
        f2T_pool = ctx.enter_context(tc.tile_pool(name="f2T", bufs=10))
        g2T_pool = ctx.enter_context(tc.tile_pool(name="g2T", bufs=3))
        h_pool = ctx.enter_context(tc.tile_pool(name="hkb", bufs=10))
        es_pool = ctx.enter_context(tc.tile_pool(name="es", bufs=14))
        yT_pool = ctx.enter_context(tc.tile_pool(name="yT", bufs=6))
        rz_pool = ctx.enter_context(tc.tile_pool(name="rz", bufs=6))
        o_pool = ctx.enter_context(tc.tile_pool(name="o", bufs=8))
        pbank = ctx.enter_context(tc.tile_pool(name="pbank", bufs=4, space="PSUM"))
        psS = ctx.enter_context(tc.tile_pool(name="psS", bufs=2, space="PSUM"))

        # per-batch tile state
        S = [dict(xg=[], xT=[], xpT=[], f2T=[], g2T=[], hk=[], es={})
             for _ in range(BPC)]

        def emit_A_load(b, qg, split=False):
            """Issue the cast-load DMA for one q-group. split=True loads the
            group as two half-DMAs into one tile with separate sub-tile
            "ready" tracking via two DMA writes -- used for the first groups
            so the transpose pipeline primes ~1.5us sooner."""
            st = S[b]
            if qg == 0:
                for cc in range(N_CC):
                    st["xT"].append(
                        xT_pool.tile([P, HW], bf16, tag="xT", name=f"xT{cc}"))
                    st["xpT"].append(
                        xpT_pool.tile([P, KP], bf16, tag="xpT", name=f"xpT{cc}"))
            xgt = xb_pool.tile([P, 4 * C], bf16, tag="xb", name=f"xb{qg}")
            src = x_ext[b, qg * 512:(qg + 1) * 512, :].rearrange(
                "(j p) c -> p j c", p=P)
            dst = xgt.rearrange("p (j c) -> p j c", j=4)
            if split:
                nc.gpsimd.dma_start(out=dst[:, 0:2, :], in_=src[:, 0:2, :])
                nc.gpsimd.dma_start(out=dst[:, 2:4, :], in_=src[:, 2:4, :])
            else:
                nc.gpsimd.dma_start(out=dst, in_=src)
            st["xg"].append(xgt)

        def emit_A_unit(b, qg):
            """Transpose + pool + f2T for one loaded q-group."""
            st = S[b]
            xgt = st["xg"][qg]
            for cc in range(N_CC):
                pt = pbank.tile([P, 512], fp32, tag="pb", name="pb")
                for j in range(4):
                    nc.tensor.matmul(
                        pt[:, j * P:(j + 1) * P],
                        lhsT=xgt[:, j * C + cc * P:j * C + (cc + 1) * P],
                        rhs=ident[:],
                        start=True, stop=True)
                dst = st["xT"][cc][:, qg * 512:(qg + 1) * 512]
                nc.scalar.activation(dst, pt[:], AF.Copy)
                # incremental pool of this q-group: 512 q -> 128 k
                # q-span = 8 rows (h) x 64 cols (w)
                v = st["xT"][cc][:, qg * 512:(qg + 1) * 512].rearrange(
                    "p (h w2 t) -> p (h w2) t", w2=32, t=2)
                t1 = ptmp_pool.tile([P, 256], bf16, tag="ptmp", name="ptmp")
                nc.vector.tensor_add(t1[:], v[:, :, 0], v[:, :, 1])
                r2 = t1.rearrange("p (h2 t w) -> p h2 t w", t=2, w=32)
                nc.gpsimd.tensor_add(
                    st["xpT"][cc][:, qg * P:(qg + 1) * P].rearrange(
                        "p (h2 w) -> p h2 w", w=32),
                    r2[:, :, 0, :], r2[:, :, 1, :])
            # f2T for this q-span (only needs this qg's xT columns)
            qs = qg
            xT = st["xT"]
            pf = pbank.tile([P, 512], fp32, tag="pb", name="pb")
            for cc in range(N_CC):
                nc.tensor.matmul(
                    pf[:],
                    lhsT=wf2[:, cc * P:(cc + 1) * P],
                    rhs=xT[cc][:, qs * 512:(qs + 1) * 512],
                    start=(cc == 0), stop=(cc == N_CC - 1))
            ft = f2T_pool.tile([P, 512], bf16, tag="f2T", name=f"f2T{qs}")
            nc.vector.tensor_copy(ft[:], pf[:])
            st["f2T"].append(ft)

        def emit_C_half(b, ks):
            """Projections for one k-half: g2T[ks] + h[kc 4ks..4ks+3].
            Only needs q-groups 4ks..4ks+3 pooled, so the first half can be
            emitted right after A-unit 3 -- unblocking every span's first
            four score/exp chunks four q-groups earlier."""
            st = S[b]
            xT, xpT = st["xT"], st["xpT"]
            pg = pbank.tile([P, 512], fp32, tag="pb", name="pb")
            for cc in range(N_CC):
                nc.tensor.matmul(
                    pg[:],
                    lhsT=wg2[:, cc * P:(cc + 1) * P],
                    rhs=xpT[cc][:, ks * 512:(ks + 1) * 512],
                    start=(cc == 0), stop=(cc == N_CC - 1))
            gt = g2T_pool.tile([P, 512], bf16, tag="g2T", name=f"g2T{ks}")
            nc.scalar.activation(gt[:], pg[:], AF.Copy)
            st["g2T"].append(gt)
            for pr in range(2 * ks, 2 * ks + 2):
                ph = pbank.tile([P, 2 * E], fp32, tag="pb", name="ph")
                for half in range(2):
                    kc = pr * 2 + half
                    for cc in range(N_CC):
                        nc.tensor.matmul(
                            ph[:, half * E:(half + 1) * E],
                            lhsT=xpT[cc][:, kc * P:(kc + 1) * P],
                            rhs=whb[:, cc * E:(cc + 1) * E],
                            start=(cc == 0), stop=(cc == N_CC - 1))
                ht = h_pool.tile([P, 2 * E], fp8, tag="hkb", name=f"hkb{pr}")
                st["hk"].append(ht)
                nc.vector.tensor_scalar_mul(ht[:], ph[:], 2.0)

        def emit_span_scores(b, qs, kh):
            """sT + exp for kc pairs (2kh, 2kh+1) of span qs. kh=0 only
            needs g2T[0] (first 512 keys), so it can prefetch into the
            stage-A ramp where psS and ACT are otherwise idle."""
            st = S[b]
            f2T, g2T = st["f2T"], st["g2T"]
            sdict = st["es"].setdefault(qs, {})
            for kp_i in (2 * kh, 2 * kh + 1):
                ps = psS.tile([P, 1024], fp32, tag="psS", name="psS")
                for half in range(2):
                    kc = kp_i * 2 + half
                    ks, off = kc // 4, (kc % 4) * P
                    if ROWPACK:
                        rlo = 64 * (kc % 2)
                        tp = (rlo, 0)
                        lhsT = g2T[ks][rlo:rlo + 64, off:off + P]
                        rhs = f2T[qs][rlo:rlo + 64, :]
                        nc.tensor.matmul(
                            ps[:, half * 512:(half + 1) * 512],
                            lhsT=lhsT, rhs=rhs,
                            start=True, stop=True, tile_position=tp)
                    else:
                        nc.tensor.matmul(
                            ps[:, half * 512:(half + 1) * 512],
                            lhsT=g2T[ks][0:64, off:off + P],
                            rhs=f2T[qs][0:64, :],
                            start=True, stop=True)
                et = es_pool.tile([P, 1024], fp8, tag="es", name="es")
                nc.scalar.activation(et[:], ps[:], AF.Exp,
                                     bias=ebias[:])
                sdict[kp_i] = et

        def emit_span(b, qs, pre_kh0=False):
            st = S[b]
            hk, xg = st["hk"], st["xg"]
            if True:
                if not pre_kh0:
                    emit_span_scores(b, qs, 0)
                emit_span_scores(b, qs, 1)
                es = [st["es"][qs][i] for i in range(4)]
                del st["es"][qs]

                # D3: Z[q] per q-chunk via matmul(lhsT=exp chunk, rhs=ones).
                # Plain fp8 (not DoubleRow): at FD=1 these are LDWEIGHTS-bound
                # and FWL (4x fp8 weight load) beats DoubleRow's 2x-wide
                # FWL-less load.
                pz = pbank.tile([P, 4], fp32, tag="pb", name="pz")
                for kc in range(N_KC):
                    for q4 in range(4):
                        lhsT = es[kc // 2][:, (kc % 2) * 512 + q4 * P:
                                           (kc % 2) * 512 + (q4 + 1) * P]
                        nc.tensor.matmul(
                            pz[:, q4:q4 + 1], lhsT=lhsT,
                            rhs=ones[:, 0:1],
                            start=(kc == 0), stop=(kc == N_KC - 1))
                rz = rz_pool.tile([P, 4], fp32, tag="rz", name="rz")
                nc.vector.reciprocal(rz[:], pz[:])

                # D4: yT[e, q_span] = h^T @ expsT  (fp8 DoubleRow, k pairs)
                yt = yT_pool.tile([P, 1024], fp8, tag="yT", name="yT")
                for ec in range(2):
                    py = pbank.tile([P, 512], fp32, tag="pb", name="pb")
                    for pr in range(4):
                        h3 = hk[pr].rearrange("p (ko e) -> p ko e", ko=2)
                        e3 = es[pr].rearrange("p (ko q) -> p ko q", ko=2)
                        nc.tensor.matmul(
                            py[:],
                            lhsT=h3[:, :, ec * P:(ec + 1) * P],
                            rhs=e3[:, :, :],
                            start=(pr == 0), stop=(pr == 3),
                            perf_mode=mybir.MatmulPerfMode.DoubleRow)
                    nc.vector.tensor_scalar_mul(
                        yt[:, ec * 512:(ec + 1) * 512], py[:], 0.25)

                # D5+D6: out[q, c] = (yT^T @ Wo) * (1/Z) + x, then DMA out
                y3 = yt.rearrange("p (ko q) -> p ko q", ko=2)
                w3 = wob.rearrange("p (ko c) -> p ko c", ko=2)
                for q4 in range(4):
                    qc = qs * 4 + q4
                    po = pbank.tile([P, 512], fp32, tag="pb", name="pb")
                    nc.tensor.matmul(
                        po[:],
                        lhsT=y3[:, :, q4 * P:(q4 + 1) * P],
                        rhs=w3[:, :, :],
                        start=True, stop=True,
                        perf_mode=mybir.MatmulPerfMode.DoubleRow)
                    ot = o_pool.tile([P, C], fp32, tag="o", name="ot")
                    xres = xg[qc // 4][:, (qc % 4) * C:(qc % 4 + 1) * C]
                    nc.vector.scalar_tensor_tensor(
                        out=ot[:], in0=po[:], scalar=rz[:, q4:q4 + 1],
                        in1=xres, op0=ALU.mult, op1=ALU.add)
                    nc.sync.dma_start(
                        out=out_ext[b, qc * P:(qc + 1) * P, :], in_=ot[:])

        # software-pipelined emission: loads run 3 q-groups ahead of their
        # compute; batch 1's stage A rides inside batch 0's span loop so its
        # loads/transposes/pools fill engine gaps
        emit_A_load(0, 0, split=True)
        emit_A_load(0, 1, split=True)
        emit_A_load(0, 2, split=True)
        emit_weight_loads()
        for qg in range(8):
            if qg + 3 < 8:
                emit_A_load(0, qg + 3)
            emit_A_unit(0, qg)
            if qg == 3:
                emit_C_half(0, 0)
        emit_C_half(0, 1)
        emit_A_load(1, 0)
        emit_A_load(1, 1)
        for qs in range(N_SPAN):
            if qs + 2 < N_SPAN:
                emit_A_load(1, qs + 2)
            emit_A_unit(1, qs)
            if qs == 3:
                emit_C_half(1, 0)
            if qs == 7:
                emit_C_half(1, 1)
            emit_span(0, qs)
        for qs in range(N_SPAN):
            emit_span(1, qs)

    nc.compile()
    return nc


_NC_CACHE = {}


def _get_nc():
    if "nc" not in _NC_CACHE:
        _NC_CACHE["nc"] = build_nc()
    return _NC_CACHE["nc"]


def _make_in_maps(inputs):
    x = np.ascontiguousarray(np.asarray(inputs["x"], dtype=np.float32))
    Wf = np.asarray(inputs["Wf"], dtype=np.float32)
    Wg = np.asarray(inputs["Wg"], dtype=np.float32)
    Wh = np.asarray(inputs["Wh"], dtype=np.float32)
    Wo = np.asarray(inputs["Wo"], dtype=np.float32)

    xr = x.reshape(B, HW, C)
    wf2 = np.ascontiguousarray(np.concatenate([Wf, Wf], axis=1))
    wg2 = np.ascontiguousarray(np.concatenate([Wg, Wg], axis=1) * 0.25)
    whq = np.ascontiguousarray(Wh * 0.25)
    wo = np.ascontiguousarray(Wo * 16.0)

    ident = np.eye(P, dtype=np.float32)
    return [
        {"x": np.ascontiguousarray(xr[i * BPC:(i + 1) * BPC]),
         "wf2": wf2, "wg2": wg2, "wh": whq, "wo": wo, "ident": ident}
        for i in range(NCORES)
    ]


def run(inputs, trace=False, **kw):
    from concourse.bass_utils import run_bass_kernel_spmd
    nc = _get_nc()
    in_maps = _make_in_maps(inputs)
    res = run_bass_kernel_spmd(nc, in_maps, core_ids=list(range(NCORES)),
                               trace=trace, **kw)
    out = np.concatenate([r["out"] for r in res.results], axis=0)
    return out.reshape(B, H, W, C).astype(np.float32), res


def kernel(**inputs):
    out, _ = run(inputs, trace=False)
    return out


# revision 86
# speedup vs baseline: 1.0418x; 1.0088x over previous
"""Self-attention (SAGAN-style) Trainium2 kernel, data-parallel over batch on
8 NeuronCores (2 images per core, no collectives).

Reference computation per batch image (B=16, H=W=64, C=512):
    f = x @ Wf                         [4096, 64]   queries
    xp = avgpool2x2(x)                 [1024, 512]
    g = xp @ Wg                        [1024, 64]   keys
    h = xp @ Wh                        [1024, 256]  values
    a = softmax(f @ g^T, axis=-1)      [4096, 1024]
    out = (a @ h) @ Wo + x             [4096, 512]

Per-core dataflow (software-pipelined across the 2 images):
  - x cast-loaded f32->bf16 by SWDGE DMA in [128, 2048] groups, PE-transposed
    (regular matmul vs identity, bf16) to xT [c,q]; 2x2 sum-pooling runs
    incrementally per q-group via strided adds (w-pairs on DVE, h-pairs on
    GPSIMD); Wg/Wh are pre-scaled 0.25 on host so sum-pool == avg-pool.
  - Projections (bf16): f2T [d dup2, q] (lhsT = [Wf|Wf]), g2T [d dup2, k],
    h [k, e]. The d=64 score matmuls are row-packed two-at-a-time into the
    128x128 PE array via tile_position (the duplication feeds rows 64-127).
  - Scores sT = g2T^T f2T accumulate in [k, q] layout; exp on ACT reads PSUM
    directly and writes fp8e4 with a free bias of -4*ln2 (softmax-invariant,
    keeps exp outputs inside fp8e4's +-240 range; no max-subtraction needed
    since |s| <= ~6.2).
  - Z[q] = sum_k exp via matmul(lhsT=exp chunk, rhs=const[128,1]) accumulated
    over k chunks -- lands [q-partition, 1], the orientation the epilogue
    needs. The const is 8.0 = alpha*beta*gamma, pre-compensating the fp8
    scale factors below so no extra scaling op exists anywhere.
  - yT = h^T exp and out_pre = yT^T Wo both run as fp8e4 DoubleRow matmuls
    (2 fp8 weights/cell, 2x MACs): h is evacuated as 2*h (alpha), yT as
    0.25*yT (gamma), Wo is host-scaled 16x (beta) to center fp8 dynamic
    range; all three factors cancel exactly through 1/Z.
  - Epilogue: one DVE scalar_tensor_tensor does out = po * (1/Z) + x.
  - Batch 1's load/transpose/pool units are emitted inside batch 0's span
    loop so they fill engine gaps (engines execute their streams in order).
"""

import numpy as np

B, H, W, C = 16, 64, 64, 512
NCORES = 8
BPC = B // NCORES          # batches per core
HW = H * W                 # 4096 queries
KP = HW // 4               # 1024 pooled keys
D2 = 128                   # duplicated query/key dim (2 x 64)
E = C // 2                 # 256 value dim
P = 128

N_QC = HW // P             # 32 q chunks of 128
N_SPAN = 8                 # q spans of 512
N_CC = C // P              # 4 channel chunks
N_KC = KP // P             # 8 key chunks

ROWPACK = True


def build_nc():
    from contextlib import ExitStack
    import concourse.bacc as bacc
    import concourse.mybir as mybir
    from concourse.tile import TileContext

    fp32 = mybir.dt.float32
    bf16 = mybir.dt.bfloat16
    fp8 = mybir.dt.float8e4
    AF = mybir.ActivationFunctionType
    ALU = mybir.AluOpType

    nc = bacc.Bacc("TRN2", target_bir_lowering=False, debug=False,
                   num_devices=NCORES)
    x_ext = nc.dram_tensor("x", [BPC, HW, C], fp32, kind="ExternalInput").ap()
    wf2_ext = nc.dram_tensor("wf2", [C, P], fp32, kind="ExternalInput").ap()
    wg2_ext = nc.dram_tensor("wg2", [C, P], fp32, kind="ExternalInput").ap()
    wh_ext = nc.dram_tensor("wh", [C, E], fp32, kind="ExternalInput").ap()
    wo_ext = nc.dram_tensor("wo", [E, C], fp32, kind="ExternalInput").ap()
    ident_ext = nc.dram_tensor("ident", [P, P], fp32, kind="ExternalInput").ap()
    out_ext = nc.dram_tensor("out", [BPC, HW, C], fp32, kind="ExternalOutput").ap()

    with ExitStack() as ctx:
        tc = ctx.enter_context(TileContext(nc))

        const = ctx.enter_context(tc.tile_pool(name="const", bufs=1))
        ident = const.tile([P, P], bf16)
        ident_f = const.tile([P, P], fp32)
        nc.sync.dma_start(out=ident_f[:], in_=ident_ext[:])
        nc.vector.tensor_copy(ident[:], ident_f[:])
        ones = const.tile([P, 2], fp8)
        nc.vector.memset(ones[:], 8.0)
        ebias = const.tile([P, 1], fp32)
        nc.vector.memset(ebias[:], -2.772588722239781)
        gamma = const.tile([P, 1], fp32)
        nc.vector.memset(gamma[:], 0.25)

        wf2 = const.tile([P, 4 * P], bf16)
        wg2 = const.tile([P, 4 * P], bf16)
        whb = const.tile([P, 4 * E], bf16)
        wob = const.tile([P, 2 * C], fp8)
        wst_pool = ctx.enter_context(tc.tile_pool(name="wst", bufs=4))

        def wload(dst_slice, src_slice, n):
            st = wst_pool.tile([P, n], fp32, tag="wst", name="wst")
            nc.sync.dma_start(out=st[:], in_=src_slice)
            nc.vector.tensor_copy(dst_slice, st[:])

        def emit_weight_loads():
            for cc in range(N_CC):
                wload(wf2[:, cc * P:(cc + 1) * P],
                      wf2_ext[cc * P:(cc + 1) * P, :], P)
                wload(wg2[:, cc * P:(cc + 1) * P],
                      wg2_ext[cc * P:(cc + 1) * P, :], P)
                wload(whb[:, cc * E:(cc + 1) * E],
                      wh_ext[cc * P:(cc + 1) * P, :], E)
            for ec in range(2):
                wload(wob[:, ec * C:(ec + 1) * C],
                      wo_ext[ec * P:(ec + 1) * P, :], C)

        xb_pool = ctx.enter_context(tc.tile_pool(name="xb", bufs=16))
        xT_pool = ctx.enter_context(tc.tile_pool(name="xT", bufs=5))
        xpT_pool = ctx.enter_context(tc.tile_pool(name="xpT", bufs=5))
        ptmp_pool = ctx.enter_context(tc.tile_pool(name="ptmp", bufs=4))
        f2T_pool = ctx.enter_context(tc.tile_pool(name="f2T", bufs=10))
        g2T_pool = ctx.enter_context(tc.tile_pool(name="g2T", bufs=3))
        h_pool = ctx.enter_context(tc.tile_pool(name="hkb", bufs=10))
        es_pool = ctx.enter_context(tc.tile_pool(name="es", bufs=14))
        yT_pool = ctx.enter_context(tc.tile_pool(name="yT", bufs=6))
        rz_pool = ctx.enter_context(tc.tile_pool(name="rz", bufs=6))
        o_pool = ctx.enter_context(tc.tile_pool(name="o", bufs=8))
        pbank = ctx.enter_context(tc.tile_pool(name="pbank", bufs=4, space="PSUM"))
        psS = ctx.enter_context(tc.tile_pool(name="psS", bufs=2, space="PSUM"))

        # per-batch tile state
        S = [dict(xg=[], xT=[], xpT=[], f2T=[], g2T=[], hk=[], es={})
             for _ in range(BPC)]

        def emit_A_load(b, qg, split=False):
            """Issue the cast-load DMA for one q-group. split=True loads the
            group as two half-DMAs into one tile with separate sub-tile
            "ready" tracking via two DMA writes -- used for the first groups
            so the transpose pipeline primes ~1.5us sooner."""
            st = S[b]
            if qg == 0:
                for cc in range(N_CC):
                    st["xT"].append(
                        xT_pool.tile([P, HW], bf16, tag="xT", name=f"xT{cc}"))
                    st["xpT"].append(
                        xpT_pool.tile([P, KP], bf16, tag="xpT", name=f"xpT{cc}"))
            xgt = xb_pool.tile([P, 4 * C], bf16, tag="xb", name=f"xb{qg}")
            src = x_ext[b, qg * 512:(qg + 1) * 512, :].rearrange(
                "(j p) c -> p j c", p=P)
            dst = xgt.rearrange("p (j c) -> p j c", j=4)
            if split:
                nc.gpsimd.dma_start(out=dst[:, 0:2, :], in_=src[:, 0:2, :])
                nc.gpsimd.dma_start(out=dst[:, 2:4, :], in_=src[:, 2:4, :])
            else:
                nc.gpsimd.dma_start(out=dst, in_=src)
            st["xg"].append(xgt)

        def emit_A_unit(b, qg):
            """Transpose + pool + f2T for one loaded q-group."""
            st = S[b]
            xgt = st["xg"][qg]
            for cc in range(N_CC):
                pt = pbank.tile([P, 512], fp32, tag="pb", name="pb")
                for j in range(4):
                    nc.tensor.matmul(
                        pt[:, j * P:(j + 1) * P],
                        lhsT=xgt[:, j * C + cc * P:j * C + (cc + 1) * P],
                        rhs=ident[:],
                        start=True, stop=True)
                dst = st["xT"][cc][:, qg * 512:(qg + 1) * 512]
                nc.scalar.activation(dst, pt[:], AF.Copy)
                # incremental pool of this q-group: 512 q -> 128 k
                # q-span = 8 rows (h) x 64 cols (w)
                v = st["xT"][cc][:, qg * 512:(qg + 1) * 512].rearrange(
                    "p (h w2 t) -> p (h w2) t", w2=32, t=2)
                t1 = ptmp_pool.tile([P, 256], bf16, tag="ptmp", name="ptmp")
                nc.vector.tensor_add(t1[:], v[:, :, 0], v[:, :, 1])
                r2 = t1.rearrange("p (h2 t w) -> p h2 t w", t=2, w=32)
                nc.gpsimd.tensor_add(
                    st["xpT"][cc][:, qg * P:(qg + 1) * P].rearrange(
                        "p (h2 w) -> p h2 w", w=32),
                    r2[:, :, 0, :], r2[:, :, 1, :])
            # f2T for this q-span (only needs this qg's xT columns)
            qs = qg
            xT = st["xT"]
            pf = pbank.tile([P, 512], fp32, tag="pb", name="pb")
            for cc in range(N_CC):
                nc.tensor.matmul(
                    pf[:],
                    lhsT=wf2[:, cc * P:(cc + 1) * P],
                    rhs=xT[cc][:, qs * 512:(qs + 1) * 512],
                    start=(cc == 0), stop=(cc == N_CC - 1))
            ft = f2T_pool.tile([P, 512], bf16, tag="f2T", name=f"f2T{qs}")
            nc.vector.tensor_copy(ft[:], pf[:])
            st["f2T"].append(ft)

        def emit_C_half(b, ks):
            """Projections for one k-half: g2T[ks] + h[kc 4ks..4ks+3].
            Only needs q-groups 4ks..4ks+3 pooled, so the first half can be
            emitted right after A-unit 3 -- unblocking every span's first
            four score/exp chunks four q-groups earlier."""
            st = S[b]
            xT, xpT = st["xT"], st["xpT"]
            pg = pbank.tile([P, 512], fp32, tag="pb", name="pb")
            for cc in range(N_CC):
                nc.tensor.matmul(
                    pg[:],
                    lhsT=wg2[:, cc * P:(cc + 1) * P],
                    rhs=xpT[cc][:, ks * 512:(ks + 1) * 512],
                    start=(cc == 0), stop=(cc == N_CC - 1))
            gt = g2T_pool.tile([P, 512], bf16, tag="g2T", name=f"g2T{ks}")
            nc.scalar.activation(gt[:], pg[:], AF.Copy)
            st["g2T"].append(gt)
            for pr in range(2 * ks, 2 * ks + 2):
                ph = pbank.tile([P, 2 * E], fp32, tag="pb", name="ph")
                for half in range(2):
                    kc = pr * 2 + half
                    for cc in range(N_CC):
                        nc.tensor.matmul(
                            ph[:, half * E:(half + 1) * E],
                            lhsT=xpT[cc][:, kc * P:(kc + 1) * P],
                            rhs=whb[:, cc * E:(cc + 1) * E],
                            start=(cc == 0), stop=(cc == N_CC - 1))
                ht = h_pool.tile([P, 2 * E], fp8, tag="hkb", name=f"hkb{pr}")
                st["hk"].append(ht)
                nc.vector.tensor_scalar_mul(ht[:], ph[:], 2.0)

        def emit_span_scores(b, qs, kh):
            """sT + exp for kc pairs (2kh, 2kh+1) of span qs. kh=0 only
            needs g2T[0] (first 512 keys), so it can prefetch into the
            stage-A ramp where psS and ACT are otherwise idle."""
            st = S[b]
            f2T, g2T = st["f2T"], st["g2T"]
            sdict = st["es"].setdefault(qs, {})
            for kp_i in (2 * kh, 2 * kh + 1):
                ps = psS.tile([P, 1024], fp32, tag="psS", name="psS")
                for half in range(2):
                    kc = kp_i * 2 + half
                    ks, off = kc // 4, (kc % 4) * P
                    if ROWPACK:
                        rlo = 64 * (kc % 2)
                        tp = (rlo, 0)
                        lhsT = g2T[ks][rlo:rlo + 64, off:off + P]
                        rhs = f2T[qs][rlo:rlo + 64, :]
                        nc.tensor.matmul(
                            ps[:, half * 512:(half + 1) * 512],
                            lhsT=lhsT, rhs=rhs,
                            start=True, stop=True, tile_position=tp)
                    else:
                        nc.tensor.matmul(
                            ps[:, half * 512:(half + 1) * 512],
                            lhsT=g2T[ks][0:64, off:off + P],
                            rhs=f2T[qs][0:64, :],
                            start=True, stop=True)
                et = es_pool.tile([P, 1024], fp8, tag="es", name="es")
                nc.scalar.activation(et[:], ps[:], AF.Exp,
                                     bias=ebias[:])
                sdict[kp_i] = et

        def emit_span(b, qs, pre_kh0=False):
            st = S[b]
            hk, xg = st["hk"], st["xg"]
            if True:
                if not pre_kh0:
                    emit_span_scores(b, qs, 0)
                emit_span_scores(b, qs, 1)
                es = [st["es"][qs][i] for i in range(4)]
                del st["es"][qs]

                # D3: Z[q] per q-chunk via matmul(lhsT=exp chunk, rhs=ones).
                # Plain fp8 (not DoubleRow): at FD=1 these are LDWEIGHTS-bound
                # and FWL (4x fp8 weight load) beats DoubleRow's 2x-wide
                # FWL-less load.
                pz = pbank.tile([P, 4], fp32, tag="pb", name="pz")
                for kc in range(N_KC):
                    for q4 in range(4):
                        lhsT = es[kc // 2][:, (kc % 2) * 512 + q4 * P:
                                           (kc % 2) * 512 + (q4 + 1) * P]
                        nc.tensor.matmul(
                            pz[:, q4:q4 + 1], lhsT=lhsT,
                            rhs=ones[:, 0:1],
                            start=(kc == 0), stop=(kc == N_KC - 1))
                rz = rz_pool.tile([P, 4], fp32, tag="rz", name="rz")
                nc.vector.reciprocal(rz[:], pz[:])

                # D4: yT[e, q_span] = h^T @ expsT  (fp8 DoubleRow, k pairs)
                yt = yT_pool.tile([P, 1024], fp8, tag="yT", name="yT")
                for ec in range(2):
                    py = pbank.tile([P, 512], fp32, tag="pb", name="pb")
                    for pr in range(4):
                        h3 = hk[pr].rearrange("p (ko e) -> p ko e", ko=2)
                        e3 = es[pr].rearrange("p (ko q) -> p ko q", ko=2)
                        nc.tensor.matmul(
                            py[:],
                            lhsT=h3[:, :, ec * P:(ec + 1) * P],
                            rhs=e3[:, :, :],
                            start=(pr == 0), stop=(pr == 3),
                            perf_mode=mybir.MatmulPerfMode.DoubleRow)
                    if b == BPC - 1 and qs >= N_SPAN - 2:
                        nc.scalar.activation(
                            yt[:, ec * 512:(ec + 1) * 512], py[:], AF.Copy,
                            scale=gamma[:])
                    else:
                        nc.vector.tensor_scalar_mul(
                            yt[:, ec * 512:(ec + 1) * 512], py[:], 0.25)

                # D5+D6: out[q, c] = (yT^T @ Wo) * (1/Z) + x, then DMA out
                y3 = yt.rearrange("p (ko q) -> p ko q", ko=2)
                w3 = wob.rearrange("p (ko c) -> p ko c", ko=2)
                for q4 in range(4):
                    qc = qs * 4 + q4
                    po = pbank.tile([P, 512], fp32, tag="pb", name="pb")
                    nc.tensor.matmul(
                        po[:],
                        lhsT=y3[:, :, q4 * P:(q4 + 1) * P],
                        rhs=w3[:, :, :],
                        start=True, stop=True,
                        perf_mode=mybir.MatmulPerfMode.DoubleRow)
                    ot = o_pool.tile([P, C], fp32, tag="o", name="ot")
                    xres = xg[qc // 4][:, (qc % 4) * C:(qc % 4 + 1) * C]
                    nc.vector.scalar_tensor_tensor(
                        out=ot[:], in0=po[:], scalar=rz[:, q4:q4 + 1],
                        in1=xres, op0=ALU.mult, op1=ALU.add)
                    nc.sync.dma_start(
                        out=out_ext[b, qc * P:(qc + 1) * P, :], in_=ot[:])

        # software-pipelined emission: loads run 3 q-groups ahead of their
        # compute; batch 1's stage A rides inside batch 0's span loop so its
        # loads/transposes/pools fill engine gaps
        emit_A_load(0, 0, split=True)
        emit_A_load(0, 1, split=True)
        emit_A_load(0, 2, split=True)
        emit_weight_loads()
        for qg in range(8):
            if qg + 3 < 8:
                emit_A_load(0, qg + 3)
            emit_A_unit(0, qg)
            if qg == 3:
                emit_C_half(0, 0)
        emit_C_half(0, 1)
        emit_A_load(1, 0)
        emit_A_load(1, 1)
        for qs in range(N_SPAN):
            if qs + 2 < N_SPAN:
                emit_A_load(1, qs + 2)
            emit_A_unit(1, qs)
            if qs == 3:
                emit_C_half(1, 0)
            if qs == 7:
                emit_C_half(1, 1)
            emit_span(0, qs)
        for qs in range(N_SPAN):
            emit_span(1, qs)

    nc.compile()
    return nc


_NC_CACHE = {}


def _get_nc():
    if "nc" not in _NC_CACHE:
        _NC_CACHE["nc"] = build_nc()
    return _NC_CACHE["nc"]


def _make_in_maps(inputs):
    x = np.ascontiguousarray(np.asarray(inputs["x"], dtype=np.float32))
    Wf = np.asarray(inputs["Wf"], dtype=np.float32)
    Wg = np.asarray(inputs["Wg"], dtype=np.float32)
    Wh = np.asarray(inputs["Wh"], dtype=np.float32)
    Wo = np.asarray(inputs["Wo"], dtype=np.float32)

    xr = x.reshape(B, HW, C)
    wf2 = np.ascontiguousarray(np.concatenate([Wf, Wf], axis=1))
    wg2 = np.ascontiguousarray(np.concatenate([Wg, Wg], axis=1) * 0.25)
    whq = np.ascontiguousarray(Wh * 0.25)
    wo = np.ascontiguousarray(Wo * 16.0)

    ident = np.eye(P, dtype=np.float32)
    return [
        {"x": np.ascontiguousarray(xr[i * BPC:(i + 1) * BPC]),
         "wf2": wf2, "wg2": wg2, "wh": whq, "wo": wo, "ident": ident}
        for i in range(NCORES)
    ]


def run(inputs, trace=False, **kw):
    from concourse.bass_utils import run_bass_kernel_spmd
    nc = _get_nc()
    in_maps = _make_in_maps(inputs)
    res = run_bass_kernel_spmd(nc, in_maps, core_ids=list(range(NCORES)),
                               trace=trace, **kw)
    out = np.concatenate([r["out"] for r in res.results], axis=0)
    return out.reshape(B, H, W, C).astype(np.float32), res


def kernel(**inputs):
    out, _ = run(inputs, trace=False)
    return out
